# revision 14
# baseline (speedup 1.0000x reference)
"""Multi-head attention (B=2, Q=K=2048, H=16, D=128) with per-batch valid_len
masking, for 8 Trainium2 NeuronCores.

Sharding: head-parallel. Core c owns heads {2c, 2c+1} of BOTH batches, so every
core runs the same compiled program (2 heads at vl0 + 2 heads at vl1).

The Bass program is built inside kernel() and specialized on the runtime
valid_lens: key positions >= valid_len contribute exactly 0 to the softmax
(exp(-1e6) == 0 in fp32), so the kernel only computes the valid k-range and
relies on the runtime's zero-initialized output buffers for the masked tail
(run_bass_kernel_spmd / bass2jax pre-zero ExternalOutput buffers; kernels that
don't write every element rely on that — see concourse/bass2jax.py).

Per head (vl = its batch's valid_len, KV = ceil128(vl), KC = ceil512(vl)):
  scores   = qT.T @ kT           PE, bf16 in / fp32 PSUM out, N=512 chunks
  exp      = Exp(scores * 1/sqrt(D))  ScalarE from PSUM, accum_out = row sums
  expT     = PE transpose of exp tiles (batched 4/PSUM bank) + DVE copyback
             with bf16 cast
  ctxT     = v.T @ expT          PE, accumulate over k-tiles
  ctx      = PE transpose of ctxT + normalize by 1/rowsum fused into the
             PSUM->SBUF copy (tensor_scalar_mul)
  attn     = exp * 1/rowsum (in place, DVE), DMA'd out as one transfer/head
"""

import math
import os
import sys

sys.path.insert(0, "/opt/trn_rl_repo")

import numpy as np

B, Q, KMAX, H, D = 2, 2048, 2048, 16, 128
P = 128  # partitions
QT = Q // P  # q tiles per head
N_CORES = 8
HEADS_PER_CORE = (B * H) // N_CORES  # 4


def _ceil(x, m):
    return (x + m - 1) // m * m


def build_program(vls, repeat=1):
    """Build the per-core Bass program. vls: per-slot valid_len (len 4,
    slot order [b0h0, b0h1, b1h0, b1h1]). repeat>1 re-runs the whole
    workload (for marginal-cost timing in the test harness)."""
    import concourse.bacc as bacc
    import concourse.mybir as mybir
    import concourse.tile as tile
    from concourse.masks import make_identity
    from contextlib import ExitStack

    f32 = mybir.dt.float32
    bf16 = mybir.dt.bfloat16
    NH = len(vls)
    scale = 1.0 / math.sqrt(D)

    KVs = [_ceil(vl, P) for vl in vls]  # k extent rounded to 128 (tiles*P)
    KCs = [_ceil(vl, 512) for vl in vls]  # k extent rounded to 512 (chunks)
    maxKV = max(KVs)
    maxKC = max(KCs)

    nc = bacc.Bacc("TRN2", target_bir_lowering=False)

    qT_d = nc.dram_tensor("qT", (NH, P, Q), bf16, kind="ExternalInput")
    kT_d = nc.dram_tensor("kT", (NH, P, KMAX), bf16, kind="ExternalInput")
    v_d = nc.dram_tensor("v", (NH, KMAX, D), bf16, kind="ExternalInput")
    attn_d = nc.dram_tensor("attn", (NH, Q, KMAX), f32, kind="ExternalOutput")
    ctx_d = nc.dram_tensor("ctx", (NH, Q, D), f32, kind="ExternalOutput")

    with tile.TileContext(nc) as tc, ExitStack() as ctx:
        consts = ctx.enter_context(tc.tile_pool(name="consts", bufs=1))
        pq = ctx.enter_context(tc.tile_pool(name="pq", bufs=2))
        pk = ctx.enter_context(tc.tile_pool(name="pk", bufs=2))
        pv = ctx.enter_context(tc.tile_pool(name="pv", bufs=2))
        pexp = ctx.enter_context(tc.tile_pool(name="pexp", bufs=2))
        pexpT = ctx.enter_context(
            tc.tile_pool(name="pexpT", bufs=(maxKV // P) + 2)
        )
        pctxT = ctx.enter_context(tc.tile_pool(name="pctxT", bufs=2))
        pctxO = ctx.enter_context(tc.tile_pool(name="pctxO", bufs=2))
        pstat = ctx.enter_context(tc.tile_pool(name="pstat", bufs=4))
        # PSUM pools: scores (2 banks max) x2 + transposes x2 + ctx x2 = 8 banks
        pscore = ctx.enter_context(
            tc.tile_pool(name="pscore", bufs=2, space="PSUM")
        )
        ptp = ctx.enter_context(tc.tile_pool(name="ptp", bufs=2, space="PSUM"))
        pctx = ctx.enter_context(tc.tile_pool(name="pctx", bufs=2, space="PSUM"))

        ident = consts.tile([P, P], f32)
        make_identity(nc, ident)

        for i in [h for _ in range(repeat) for h in range(NH)]:
            vl = vls[i]
            KV = KVs[i]  # padded k extent (multiple of 128)
            KC = KCs[i]  # padded k extent (multiple of 512)
            NKT = KV // P  # number of k tiles
            pad = KV - vl  # zero-padding columns in exp tiles

            # ---- loads ----
            qT_s = pq.tile([P, Q], bf16, tag="qT")
            nc.sync.dma_start(out=qT_s, in_=qT_d[i])
            kT_s = pk.tile([P, maxKC], bf16, tag="kT")
            nc.sync.dma_start(out=kT_s[:, :KC], in_=kT_d[i][:, :KC])
            # v as (p, t*128+d) so tile t is cols [t*128, t*128+128)
            v_s = pv.tile([P, maxKV], bf16, tag="v")
            nfull = vl // P  # fully-valid k tiles
            r = vl - nfull * P  # valid rows in boundary tile
            if nfull:
                nc.sync.dma_start(
                    out=v_s[:, : nfull * P].rearrange("p (t d) -> p t d", d=D),
                    in_=v_d[i, : nfull * P].rearrange("(t p) d -> p t d", p=P),
                )
            if r:
                nc.vector.memset(v_s[:, nfull * P : nfull * P + D], 0.0)
                nc.sync.dma_start(
                    out=v_s[:r, nfull * P : nfull * P + D],
                    in_=v_d[i, nfull * P : vl],
                )

            # ---- per-q-tile: scores -> exp (+row sums) ----
            exp_s = pexp.tile([P, QT * maxKV], f32, tag="exp")
            exp3 = exp_s.rearrange("p (j c) -> p j c", j=QT)
            if pad:
                nc.vector.memset(exp3[:, :, vl:KV], 0.0)
            sums = pstat.tile([P, QT], f32, tag="sums")
            for j in range(QT):
                sc = pscore.tile([P, maxKC], f32, tag="score")
                for c in range(KC // 512):
                    nc.tensor.matmul(
                        sc[:, c * 512 : (c + 1) * 512],
                        lhsT=qT_s[:, j * P : (j + 1) * P],
                        rhs=kT_s[:, c * 512 : (c + 1) * 512],
                        start=True,
                        stop=True,
                    )
                nc.scalar.activation(
                    out=exp3[:, j, :vl],
                    in_=sc[:, :vl],
                    func=mybir.ActivationFunctionType.Exp,
                    scale=scale,
                    accum_out=sums[:, j : j + 1],
                )

            # ---- transpose exp -> expT (bf16), batched 4 q-tiles per bank ----
            expT = [
                pexpT.tile([P, Q], bf16, tag="expT", name=f"expT{t}")
                for t in range(NKT)
            ]
            for t in range(NKT):
                for jg in range(QT // 4):
                    tp = ptp.tile([P, 512], f32, tag="tp")
                    for s in range(4):
                        j = jg * 4 + s
                        nc.tensor.transpose(
                            tp[:, s * P : (s + 1) * P],
                            exp3[:, j, t * P : (t + 1) * P],
                            ident,
                        )
                    nc.vector.tensor_copy(
                        expT[t][:, jg * 512 : (jg + 1) * 512], tp
                    )

            # ---- normalize attn in place, write out ----
            recip = pstat.tile([P, QT], f32, tag="recip")
            nc.vector.reciprocal(recip, sums)
            for j in range(QT):
                nc.vector.tensor_scalar_mul(
                    exp3[:, j, :vl], exp3[:, j, :vl], recip[:, j : j + 1]
                )
            nc.sync.dma_start(
                out=attn_d[i].rearrange("(j p) k -> p j k", p=P)[:, :, :vl],
                in_=exp3[:, :, :vl],
            )

            # ---- ctxT = v.T @ expT, then transpose + normalize ----
            ctx_o = pctxO.tile([P, Q], f32, tag="ctxO")
            for qc in range(Q // 512):
                cps = pctx.tile([P, 512], f32, tag="ctx")
                for t in range(NKT):
                    nc.tensor.matmul(
                        cps,
                        lhsT=v_s[:, t * P : t * P + D],
                        rhs=expT[t][:, qc * 512 : (qc + 1) * 512],
                        start=(t == 0),
                        stop=(t == NKT - 1),
                    )
                ctxT_s = pctxT.tile([P, 512], f32, tag="ctxT")
                nc.vector.tensor_copy(ctxT_s, cps)
                tp2 = ptp.tile([P, 512], f32, tag="tp")
                for s in range(4):
                    nc.tensor.transpose(
                        tp2[:, s * P : (s + 1) * P],
                        ctxT_s[:, s * P : (s + 1) * P],
                        ident,
                    )
                for s in range(4):
                    j = qc * 4 + s
                    nc.vector.tensor_scalar_mul(
                        ctx_o[:, j * P : (j + 1) * P],
                        tp2[:, s * P : (s + 1) * P],
                        recip[:, j : j + 1],
                    )
            nc.sync.dma_start(
                out=ctx_d[i].rearrange("(j p) d -> p j d", p=P),
                in_=ctx_o.rearrange("p (j d) -> p j d", d=D),
            )

    nc.finalize()
    return nc


# test-harness hooks; the grading path leaves these at their defaults
RUN_KWARGS = {}
LAST_RESULT = None
REPEAT = 1


def kernel(qs, ks, vs, valid_lens):
    global LAST_RESULT
    import ml_dtypes

    from concourse.bass_utils import run_bass_kernel_spmd

    qs = np.asarray(qs)
    ks = np.asarray(ks)
    vs = np.asarray(vs)
    valid_lens = np.asarray(valid_lens)
    assert qs.shape == (B, Q, H, D), qs.shape
    vl0, vl1 = int(valid_lens[0]), int(valid_lens[1])

    bf = ml_dtypes.bfloat16
    # (B,H,P,Q) head-major, d-on-partitions layouts for q and k
    qT_all = np.ascontiguousarray(qs.transpose(0, 2, 3, 1)).astype(bf)
    kT_all = np.ascontiguousarray(ks.transpose(0, 2, 3, 1)).astype(bf)
    v_all = np.ascontiguousarray(vs.transpose(0, 2, 1, 3)).astype(bf)

    vls = [vl0, vl0, vl1, vl1]
    nc = build_program(vls, repeat=REPEAT)

    in_maps = []
    for c in range(N_CORES):
        h0 = 2 * c
        sel = lambda a: np.ascontiguousarray(
            a[:, h0 : h0 + 2].reshape(HEADS_PER_CORE, *a.shape[2:])
        )
        in_maps.append({"qT": sel(qT_all), "kT": sel(kT_all), "v": sel(v_all)})

    res = run_bass_kernel_spmd(
        nc, in_maps, core_ids=list(range(N_CORES)), **RUN_KWARGS
    )
    LAST_RESULT = res

    attn = np.empty((B, H, Q, KMAX), np.float32)
    ctx_bhqd = np.empty((B, H, Q, D), np.float32)
    for c in range(N_CORES):
        r = res.results[c]
        for b in range(B):
            for j in range(2):
                attn[b, 2 * c + j] = r["attn"][2 * b + j]
                ctx_bhqd[b, 2 * c + j] = r["ctx"][2 * b + j]
    ctx = np.ascontiguousarray(ctx_bhqd.transpose(0, 2, 1, 3))
    return ctx, attn


if __name__ == "__main__":
    # smoke: build only
    nc = build_program([288, 288, 576, 576])
    print("build ok:", len(nc.m.functions[0].instructions) if hasattr(nc.m.functions[0], "instructions") else "n/a")


# revision 27
# speedup vs baseline: 2341.9351x; 2341.9351x over previous
"""Multi-head attention (B=2, Q=K=2048, H=16, D=128) with per-batch valid_len
masking, for 8 Trainium2 NeuronCores.

Sharding: head-parallel. Core c owns heads {2c, 2c+1} of BOTH batches, so every
core runs the same compiled program (2 heads at vl0 + 2 heads at vl1).

The Bass program is built inside kernel() and specialized on the runtime
valid_lens: key positions >= valid_len contribute exactly 0 to the softmax
(exp(-1e6) == 0 in fp32), so the kernel only computes the valid k-range and
relies on the runtime's zero-initialized output buffers for the masked tail
(run_bass_kernel_spmd / bass2jax pre-zero ExternalOutput buffers; kernels that
don't write every element rely on that — see concourse/bass2jax.py).

Per head (vl = its batch's valid_len, KV = ceil128(vl), KC = ceil512(vl)):
  scores   = qT.T @ kT           PE, bf16 in / fp32 PSUM out, N=512 chunks
  exp      = Exp(scores * 1/sqrt(D))  ScalarE from PSUM, accum_out = row sums
  expT     = PE transpose of exp tiles (batched 4/PSUM bank) + DVE copyback
             with bf16 cast
  ctxT     = v.T @ expT          PE, accumulate over k-tiles
  ctx      = PE transpose of ctxT + normalize by 1/rowsum fused into the
             PSUM->SBUF copy (tensor_scalar_mul)
  attn     = exp * 1/rowsum (in place, DVE), DMA'd out as one transfer/head
"""

import math
import os
import sys

sys.path.insert(0, "/opt/trn_rl_repo")

import numpy as np

B, Q, KMAX, H, D = 2, 2048, 2048, 16, 128
P = 128  # partitions
QT = Q // P  # q tiles per head
N_CORES = 8
HEADS_PER_CORE = (B * H) // N_CORES  # 4


def _ceil(x, m):
    return (x + m - 1) // m * m


def build_program(vls, repeat=1, loop_n=0):
    """Build the per-core Bass program. vls: per-slot valid_len (len 4,
    slot order [b0h0, b0h1, b1h0, b1h1]). repeat>1 re-runs the whole
    workload inline; loop_n>0 wraps it in a hardware For_i loop (both only
    used for marginal-cost timing in the test harness)."""
    import concourse.bacc as bacc
    import concourse.mybir as mybir
    import concourse.tile as tile
    from concourse.masks import make_identity
    from contextlib import ExitStack

    f32 = mybir.dt.float32
    bf16 = mybir.dt.bfloat16
    NH = len(vls)
    scale = 1.0 / math.sqrt(D)

    KVs = [_ceil(vl, P) for vl in vls]  # k extent rounded to 128 (tiles*P)
    KCs = [_ceil(vl, 512) for vl in vls]  # k extent rounded to 512 (chunks)
    maxKV = max(KVs)
    maxKC = max(KCs)

    nc = bacc.Bacc("TRN2", target_bir_lowering=False)

    qT_d = nc.dram_tensor("qT", (NH, P, Q), bf16, kind="ExternalInput")
    kT_d = nc.dram_tensor("kT", (NH, P, KMAX), bf16, kind="ExternalInput")
    v_d = nc.dram_tensor("v", (NH, KMAX, D), bf16, kind="ExternalInput")
    attn_d = nc.dram_tensor("attn", (NH, Q, KMAX), f32, kind="ExternalOutput")
    ctx_d = nc.dram_tensor("ctx", (NH, Q, D), f32, kind="ExternalOutput")

    with tile.TileContext(nc) as tc, ExitStack() as ctx:
        consts = ctx.enter_context(tc.tile_pool(name="consts", bufs=1))
        pq = ctx.enter_context(tc.tile_pool(name="pq", bufs=2))
        pk = ctx.enter_context(tc.tile_pool(name="pk", bufs=2))
        pv = ctx.enter_context(tc.tile_pool(name="pv", bufs=2))
        pexp = ctx.enter_context(tc.tile_pool(name="pexp", bufs=2))
        pexpT = ctx.enter_context(
            tc.tile_pool(name="pexpT", bufs=2 * (maxKV // P) + 2)
        )
        pctxO = ctx.enter_context(tc.tile_pool(name="pctxO", bufs=2))
        pstat = ctx.enter_context(tc.tile_pool(name="pstat", bufs=4))
        # PSUM pools: scores (2 banks max) x2 + transposes x2 + ctx x2 = 8 banks
        pscore = ctx.enter_context(
            tc.tile_pool(name="pscore", bufs=2, space="PSUM")
        )
        ptp = ctx.enter_context(tc.tile_pool(name="ptp", bufs=2, space="PSUM"))
        pctx = ctx.enter_context(tc.tile_pool(name="pctx", bufs=2, space="PSUM"))

        ident = consts.tile([P, P], f32)
        make_identity(nc, ident)

        def loads(i, split_q=False):
            vl, KV, KC = vls[i], KVs[i], KCs[i]
            qT_s = pq.tile([P, Q], bf16, tag="qT", name="qT_s")
            kT_s = pk.tile([P, maxKC], bf16, tag="kT", name="kT_s")
            if split_q:
                # first head: land k and the first q tile quickly so the
                # first matmul isn't gated on the full 1 MB of loads
                nc.sync.dma_start(out=kT_s[:, :KC], in_=kT_d[i][:, :KC])
                nc.sync.dma_start(out=qT_s[:, :2 * P], in_=qT_d[i][:, :2 * P])
                nc.sync.dma_start(out=qT_s[:, 2 * P :], in_=qT_d[i][:, 2 * P :])
            else:
                nc.sync.dma_start(out=qT_s, in_=qT_d[i])
                nc.sync.dma_start(out=kT_s[:, :KC], in_=kT_d[i][:, :KC])
            # v as (p, t*128+d) so k-tile t is cols [t*128, t*128+128)
            v_s = pv.tile([P, maxKV], bf16, tag="v", name="v_s")
            nfull = vl // P  # fully-valid k tiles
            r = vl - nfull * P  # valid rows in boundary tile
            if nfull:
                nc.sync.dma_start(
                    out=v_s[:, : nfull * P].rearrange("p (t d) -> p t d", d=D),
                    in_=v_d[i, : nfull * P].rearrange("(t p) d -> p t d", p=P),
                )
            if r:
                nc.gpsimd.memset(v_s[:, nfull * P : nfull * P + D], 0.0)
                nc.sync.dma_start(
                    out=v_s[:r, nfull * P : nfull * P + D],
                    in_=v_d[i, nfull * P : vl],
                )
            return qT_s, kT_s, v_s

        def scores_exp(i, qT_s, kT_s):
            vl, KV, KC = vls[i], KVs[i], KCs[i]
            exp_s = pexp.tile([P, QT * maxKV], f32, tag="exp", name="exp_s")
            exp3 = exp_s.rearrange("p (j c) -> p j c", j=QT)
            if KV - vl:
                nc.gpsimd.memset(exp3[:, :, vl:KV], 0.0)
            sums = pstat.tile([P, QT], f32, tag="sums", name="sums")
            for j in range(QT):
                sc = pscore.tile([P, maxKC], f32, tag="score", name="sc")
                for c in range(KC // 512):
                    nc.tensor.matmul(
                        sc[:, c * 512 : (c + 1) * 512],
                        lhsT=qT_s[:, j * P : (j + 1) * P],
                        rhs=kT_s[:, c * 512 : (c + 1) * 512],
                        start=True,
                        stop=True,
                    )
                nc.scalar.activation(
                    out=exp3[:, j, :vl],
                    in_=sc[:, :vl],
                    func=mybir.ActivationFunctionType.Exp,
                    scale=scale,
                    accum_out=sums[:, j : j + 1],
                )
            return exp3, sums

        ncopy = [0]

        def ctx_jgroup(i, jg, v_s, expT, recip, ctx_o):
            NKT = KVs[i] // P
            for j in range(jg * 4, jg * 4 + 4):
                cps = pctx.tile([P, D], f32, tag="ctx", name="cps")
                for t in range(NKT):
                    nc.tensor.matmul(
                        cps,
                        lhsT=expT[t][:, j * P : (j + 1) * P],
                        rhs=v_s[:, t * P : t * P + D],
                        start=(t == 0),
                        stop=(t == NKT - 1),
                    )
                nc.vector.tensor_scalar_mul(
                    ctx_o[:, j * P : (j + 1) * P], cps, recip[:, j : j + 1]
                )

        def ctx_dma(i, ctx_o):
            nc.sync.dma_start(
                out=ctx_d[i].rearrange("(j p) d -> p j d", p=P),
                in_=ctx_o.rearrange("p (j d) -> p j d", d=D),
            )

        def mid_phase(i, exp3, sums, v_s=None, fuse_ctx=False):
            """Transpose exp->expT (bf16), then per-j-group: reciprocal,
            normalize in place, write the attn chunk out. With fuse_ctx
            (last head), interleave the ctx accumulation per j-group to
            shorten the kernel tail."""
            vl, KV = vls[i], KVs[i]
            NKT = KV // P
            expT = [
                pexpT.tile([P, Q], bf16, tag="expT", name=f"expT{t}")
                for t in range(NKT)
            ]
            recip = pstat.tile([P, QT], f32, tag="recip", name="recip")
            attn3 = attn_d[i].rearrange("(j p) k -> p j k", p=P)
            ctx_o = (
                pctxO.tile([P, Q], f32, tag="ctxO", name="ctx_o")
                if fuse_ctx
                else None
            )
            for jg in range(QT // 4):
                for t in range(NKT):
                    tp = ptp.tile([P, 512], f32, tag="tp", name="tp")
                    for s in range(4):
                        j = jg * 4 + s
                        nc.tensor.transpose(
                            tp[:, s * P : (s + 1) * P],
                            exp3[:, j, t * P : (t + 1) * P],
                            ident,
                        )
                    # split PSUM->SBUF copybacks between ScalarE and DVE to
                    # balance engine load
                    dst = expT[t][:, jg * 512 : (jg + 1) * 512]
                    if ncopy[0] % 8 < 3:
                        nc.scalar.copy(dst, tp)
                    else:
                        nc.vector.tensor_copy(dst, tp)
                    ncopy[0] += 1
                jsl = slice(jg * 4, jg * 4 + 4)
                nc.vector.reciprocal(recip[:, jsl], sums[:, jsl])
                for s in range(4):
                    j = jg * 4 + s
                    nc.vector.tensor_scalar_mul(
                        exp3[:, j, :vl], exp3[:, j, :vl], recip[:, j : j + 1]
                    )
                nc.sync.dma_start(
                    out=attn3[:, jsl, :vl], in_=exp3[:, jsl, :vl]
                )
                if fuse_ctx:
                    ctx_jgroup(i, jg, v_s, expT, recip, ctx_o)
            if fuse_ctx:
                ctx_dma(i, ctx_o)
            return expT, recip

        def ctx_phase(i, v_s, expT, recip):
            ctx_o = pctxO.tile([P, Q], f32, tag="ctxO", name="ctx_o")
            for jg in range(QT // 4):
                ctx_jgroup(i, jg, v_s, expT, recip, ctx_o)
            ctx_dma(i, ctx_o)

        # big heads first; 1-head skew: ctx of head i is emitted after
        # scores+exp of head i+1 so ACT never starves behind PE's tail work
        def body():
            order = sorted(range(NH), key=lambda i: -vls[i]) * repeat
            deferred = None
            for n, i in enumerate(order):
                last = n == len(order) - 1
                qT_s, kT_s, v_s = loads(i, split_q=(n == 0 and not loop_n))
                exp3, sums = scores_exp(i, qT_s, kT_s)
                if deferred is not None:
                    ctx_phase(*deferred)
                expT, recip = mid_phase(i, exp3, sums, v_s=v_s, fuse_ctx=last)
                deferred = None if last else (i, v_s, expT, recip)
            if deferred is not None:
                ctx_phase(*deferred)

        if loop_n:
            with tc.For_i(0, loop_n, 1):
                body()
        else:
            body()

    nc.finalize()
    return nc


# test-harness hooks; the grading path leaves these at their defaults
RUN_KWARGS = {}
LAST_RESULT = None
REPEAT = 1
LOOP_N = 0


def kernel(qs, ks, vs, valid_lens):
    global LAST_RESULT
    import ml_dtypes

    from concourse.bass_utils import run_bass_kernel_spmd

    qs = np.asarray(qs)
    ks = np.asarray(ks)
    vs = np.asarray(vs)
    valid_lens = np.asarray(valid_lens)
    assert qs.shape == (B, Q, H, D), qs.shape
    vl0, vl1 = int(valid_lens[0]), int(valid_lens[1])

    bf = ml_dtypes.bfloat16
    # (B,H,P,Q) head-major, d-on-partitions layouts for q and k
    qT_all = np.ascontiguousarray(qs.transpose(0, 2, 3, 1)).astype(bf)
    kT_all = np.ascontiguousarray(ks.transpose(0, 2, 3, 1)).astype(bf)
    v_all = np.ascontiguousarray(vs.transpose(0, 2, 1, 3)).astype(bf)

    vls = [vl0, vl0, vl1, vl1]
    nc = build_program(vls, repeat=REPEAT, loop_n=LOOP_N)

    in_maps = []
    for c in range(N_CORES):
        h0 = 2 * c
        sel = lambda a: np.ascontiguousarray(
            a[:, h0 : h0 + 2].reshape(HEADS_PER_CORE, *a.shape[2:])
        )
        in_maps.append({"qT": sel(qT_all), "kT": sel(kT_all), "v": sel(v_all)})

    res = run_bass_kernel_spmd(
        nc, in_maps, core_ids=list(range(N_CORES)), **RUN_KWARGS
    )
    LAST_RESULT = res

    attn = np.empty((B, H, Q, KMAX), np.float32)
    ctx_bhqd = np.empty((B, H, Q, D), np.float32)
    for c in range(N_CORES):
        r = res.results[c]
        for b in range(B):
            for j in range(2):
                attn[b, 2 * c + j] = r["attn"][2 * b + j]
                ctx_bhqd[b, 2 * c + j] = r["ctx"][2 * b + j]
    ctx = np.ascontiguousarray(ctx_bhqd.transpose(0, 2, 1, 3))
    return ctx, attn


if __name__ == "__main__":
    # smoke: build only
    nc = build_program([288, 288, 576, 576])
    print("build ok:", len(nc.m.functions[0].instructions) if hasattr(nc.m.functions[0], "instructions") else "n/a")
